# revision 3
# baseline (speedup 1.0000x reference)
"""Trainium2 Bass kernel for nn_DAGExecutor: soft-digit decode + log/linear DAG.

Sharding: pure data-parallel. B=8 batches -> one batch (2048 tokens) per core.
Per-core layout: tokens on partitions, 16 chunk-columns of 128 tokens each.
"""
import sys
for _p in ("/opt/trn_rl_repo", "/root/.axon_site/_ro/trn_rl_repo"):
    if _p not in sys.path:
        sys.path.insert(0, _p)
import numpy as np

B, T, DEPTH = 8, 2048, 8
NI = DEPTH + 1           # 9 initial nodes
TN = 2 * DEPTH + 1       # 17 total nodes
D = 8                    # digit positions
BASE = 10
NG = NI * D              # 72 groups per token
C = 16                   # chunk columns
P = 128                  # partitions (tokens per chunk)
LN1EM12 = float(np.log(np.float32(1e-12)))

_CACHE = {}


def _build():
    import concourse.bacc as bacc
    import concourse.tile as tile
    from concourse import mybir

    f32 = mybir.dt.float32
    Alu = mybir.AluOpType
    Act = mybir.ActivationFunctionType
    Ax = mybir.AxisListType

    nc = bacc.Bacc("TRN2", target_bir_lowering=False, debug=False,
                   enable_asserts=True, num_devices=8)
    x_d = nc.dram_tensor("x", (T, NG * BASE), f32, kind="ExternalInput").ap()
    v_d = nc.dram_tensor("vs", (T, TN), f32, kind="ExternalInput").ap()
    o_d = nc.dram_tensor("o", (T, DEPTH, TN), f32, kind="ExternalInput").ap()
    g_d = nc.dram_tensor("g", (T, DEPTH), f32, kind="ExternalInput").ap()
    vpat_d = nc.dram_tensor("vpat", (P, BASE), f32, kind="ExternalInput").ap()
    pow_d = nc.dram_tensor("powt", (P, D), f32, kind="ExternalInput").ap()
    out_d = nc.dram_tensor("out", (T,), f32, kind="ExternalOutput").ap()

    with tile.TileContext(nc) as tc:
        _kernel_body(nc, tc, mybir, f32, Alu, Act, Ax,
                     x_d, v_d, o_d, g_d, vpat_d, pow_d, out_d)
    nc.compile()
    return nc


def _kernel_body(nc, tc, mybir, f32, Alu, Act, Ax,
                 x_d, v_d, o_d, g_d, vpat_d, pow_d, out_d):
    from contextlib import ExitStack
    ctx = ExitStack()
    with ctx:
        pool = ctx.enter_context(tc.tile_pool(name="main", bufs=1))

        # ---------- loads ----------
        xt = pool.tile([P, C, NG * BASE], f32)          # digit logits, token-major
        xv = x_d.rearrange("(c p) f -> p c f", p=P)     # (128, 16, 720) view of DRAM
        for c in range(C):
            nc.sync.dma_start(xt[:, c, :], xv[:, c, :])

        ot = pool.tile([P, C, DEPTH, TN], f32)
        nc.sync.dma_start(ot[:], o_d.rearrange("(c p) s n -> p c s n", p=P))
        gt = pool.tile([P, C, DEPTH], f32)
        nc.sync.dma_start(gt[:], g_d.rearrange("(c p) s -> p c s", p=P))
        g1m = pool.tile([P, C, DEPTH], f32)
        nc.vector.tensor_scalar(g1m[:], gt[:], -1.0, 1.0, Alu.mult, Alu.add)
        vpat = pool.tile([P, BASE], f32)
        nc.sync.dma_start(vpat[:], vpat_d)
        powt = pool.tile([P, D], f32)
        nc.sync.dma_start(powt[:], pow_d)

        # state: k0=mag k1=sgn k2=logmag k3=smag
        ST = pool.tile([P, 4, C, TN], f32)
        nc.sync.dma_start(ST[:, 1, :, :], v_d.rearrange("(c p) n -> p c n", p=P))

        # ---------- digit phase (all-DVE v1) ----------
        x4 = xt[:].rearrange("p c (g v) -> p c g v", v=BASE)
        mx = pool.tile([P, C, NG], f32)
        nc.vector.tensor_reduce(mx[:], x4, axis=Ax.X, op=Alu.max)
        # y = x - mx (in place)
        nc.vector.tensor_tensor(x4, x4, mx[:, :, :, None].to_broadcast((P, C, NG, BASE)),
                                Alu.subtract)
        # e = exp(100*y) in place
        nc.scalar.activation(xt[:], xt[:], Act.Exp, scale=100.0)
        st = pool.tile([P, C, NG], f32)
        nc.vector.tensor_reduce(st[:], x4, axis=Ax.X, op=Alu.add)
        # ev = e * v (in place)
        nc.vector.tensor_tensor(x4, x4,
                                vpat[:, None, None, :].to_broadcast((P, C, NG, BASE)),
                                Alu.mult)
        svt = pool.tile([P, C, NG], f32)
        nc.vector.tensor_reduce(svt[:], x4, axis=Ax.X, op=Alu.add)
        rs = pool.tile([P, C, NG], f32)
        nc.vector.reciprocal(rs[:], st[:])
        # q = sv * rs (in place on svt)
        nc.vector.tensor_tensor(svt[:], svt[:], rs[:], Alu.mult)
        # qp = q * pow
        q4 = svt[:].rearrange("p c (n d) -> p c n d", d=D)
        nc.vector.tensor_tensor(q4, q4,
                                powt[:, None, None, :].to_broadcast((P, C, NI, D)),
                                Alu.mult)
        v0 = pool.tile([P, C, NI], f32)
        nc.vector.tensor_reduce(v0[:], q4, axis=Ax.X, op=Alu.add)
        # mag[0:9] = clip(v0, 1e-8, 1e6)
        nc.vector.tensor_scalar(ST[:, 0, :, 0:NI], v0[:], 1e-8, 1e6, Alu.max, Alu.min)
        nc.vector.memset(ST[:, 0, :, NI:TN], 0.0)
        # logmag
        nc.scalar.activation(ST[:, 2, :, 0:NI], ST[:, 0, :, 0:NI], Act.Ln)
        nc.vector.memset(ST[:, 2, :, NI:TN], LN1EM12)
        # smag
        nc.vector.tensor_tensor(ST[:, 3, :, 0:NI], ST[:, 0, :, 0:NI],
                                ST[:, 1, :, 0:NI], Alu.mult)
        nc.vector.memset(ST[:, 3, :, NI:TN], 0.0)

        # ---------- DAG prep ----------
        c2 = pool.tile([P, C, DEPTH, TN], f32)          # 2*|O|
        nc.scalar.activation(c2[:], ot[:], Act.Abs, scale=2.0)

        AB = pool.tile([P, 2, C], f32)
        Rsp = pool.tile([P, 2, C], f32)                 # [sp | R]
        S = pool.tile([P, 9, C], f32)   # 9 cols of 16: in(3): clipR,2e4sp,2e4R | E(3) | absR | T(2)
        tmpP = pool.tile([P, 2, C, TN], f32)
        wt16 = pool.tile([P, C, TN], f32)
        tmpz = pool.tile([P, 2, C], f32)
        tmp16 = pool.tile([P, C], f32)
        negR = pool.tile([P, C], f32)

        for s in range(DEPTH):
            L = NI + s
            idx = NI + s
            O_s = ot[:, :, s, 0:L]                      # [P, C, L]
            # P1 = O*logmag ; P2 = O*smag   (kinds 2,3 adjacent)
            nc.vector.tensor_tensor(
                tmpP[:, :, :, 0:L],
                ot[:, None, :, s, 0:L].to_broadcast((P, 2, C, L)),
                ST[:, 2:4, :, 0:L], Alu.mult)
            nc.vector.tensor_reduce(AB[:], tmpP[:, :, :, 0:L], axis=Ax.X, op=Alu.add)
            # w = c2_s * sgn ; w1 = w + 1 (gpsimd)
            nc.gpsimd.tensor_tensor(wt16[:, :, 0:L], c2[:, :, s, 0:L],
                                    ST[:, 1, :, 0:L], Alu.mult)
            nc.gpsimd.tensor_scalar(wt16[:, :, 0:L], wt16[:, :, 0:L], 1.0, None,
                                    Alu.add)
            # sp = prod(w1)  -> Rsp[:,0,:]
            nc.vector.tensor_reduce(Rsp[:, 0, :], wt16[:, :, 0:L], axis=Ax.X,
                                    op=Alu.mult)
            # R = (1-G)*A + G*B -> Rsp[:,1,:]  (direct form: inf-safe)
            G_s = gt[:, :, s]                           # [P, C]
            nc.gpsimd.tensor_tensor(tmp16[:], AB[:, 0, :], g1m[:, :, s], Alu.mult)
            nc.gpsimd.tensor_tensor(negR[:], AB[:, 1, :], G_s, Alu.mult)
            nc.gpsimd.tensor_tensor(Rsp[:, 1, :], tmp16[:], negR[:], Alu.add)
            # stage fills: S0=clipR, S[1:3]=2e4*[sp,R]
            nc.vector.tensor_scalar(S[:, 0, :], Rsp[:, 1, :], -100.0, 100.0,
                                    Alu.max, Alu.min)
            nc.vector.tensor_scalar(S[:, 1:3, :], Rsp[:], 2.0e4, None, Alu.mult)
            # E = exp(S[0:3]) -> S[3:6] = [Emag | Elogsp | Elin]
            nc.scalar.activation(S[:, 3:6, :], S[:, 0:3, :], Act.Exp)
            # absR -> S[6]
            nc.vector.tensor_scalar(negR[:], Rsp[:, 1, :], -1.0, None, Alu.mult)
            nc.vector.tensor_tensor(S[:, 6, :], Rsp[:, 1, :], negR[:], Alu.max)
            # tanh-assembly on [Elogsp|Elin] S[4:6] -> T=[Tlog|Tlin] S[7:9]
            nc.vector.tensor_scalar(S[:, 7:9, :], S[:, 4:6, :], 1.0, None, Alu.add)
            nc.vector.reciprocal(S[:, 7:9, :], S[:, 7:9, :])
            nc.vector.tensor_scalar(S[:, 7:9, :], S[:, 7:9, :], -2.0, 1.0,
                                    Alu.mult, Alu.add)
            # mix (inf-safe): out = y*(1-G) + x*G
            # pairs (m, s): x = [absR S6 | Tlin S8], y = [Emag S3 | Tlog S7]
            xAP = S[:, 6:9:2, :]                        # cols 6, 8
            yAP = S[:, 3:8:4, :]                        # cols 3, 7
            nc.vector.tensor_tensor(
                tmpz[:], yAP,
                g1m[:, None, :, s].to_broadcast((P, 2, C)), Alu.mult)
            nc.vector.tensor_tensor(
                tmpP[:, 0:2, :, 0], xAP,
                gt[:, None, :, s].to_broadcast((P, 2, C)), Alu.mult)
            nc.vector.tensor_tensor(ST[:, 0:2, :, idx], tmpz[:],
                                    tmpP[:, 0:2, :, 0], Alu.add)
            # smag_new
            nc.gpsimd.tensor_tensor(ST[:, 3, :, idx], ST[:, 0, :, idx],
                                    ST[:, 1, :, idx], Alu.mult)
            # logmag_new = ln(max(m_new, 1e-12))
            nc.vector.tensor_scalar(tmp16[:], ST[:, 0, :, idx], 1e-12, None, Alu.max)
            nc.scalar.activation(ST[:, 2, :, idx], tmp16[:], Act.Ln)

        # ---------- output ----------
        outt = pool.tile([P, C], f32)
        nc.vector.tensor_tensor(outt[:], ST[:, 1, :, TN - 1], ST[:, 0, :, TN - 1],
                                Alu.mult)
        nc.sync.dma_start(out_d.rearrange("(c p) -> p c", p=P), outt[:])


def kernel(digit_logits, V_sign, O, G):
    if "nc" not in _CACHE:
        _CACHE["nc"] = _build()
    nc = _CACHE["nc"]
    from concourse.bass_utils import run_bass_kernel_spmd

    vpat = np.broadcast_to(np.arange(BASE, dtype=np.float32), (P, BASE)).copy()
    powt = np.broadcast_to(
        np.power(np.float32(BASE),
                 (4 - 1) - np.arange(D, dtype=np.float32)).astype(np.float32),
        (P, D)).copy()
    in_maps = []
    for i in range(B):
        in_maps.append({
            "x": np.ascontiguousarray(digit_logits[i].reshape(T, NG * BASE)),
            "vs": np.ascontiguousarray(V_sign[i]),
            "o": np.ascontiguousarray(O[i]),
            "g": np.ascontiguousarray(G[i]),
            "vpat": vpat,
            "powt": powt,
        })
    res = run_bass_kernel_spmd(nc, in_maps, core_ids=list(range(B)))
    out = np.stack([res.results[i]["out"] for i in range(B)], axis=0)
    return out.astype(np.float32)
